# revision 14
# baseline (speedup 1.0000x reference)
"""Trainium2 Bass kernel for nn_DenseTf: out = inputs @ sign(clip(w,-1,1)) + b.

Shapes: inputs [8192, 2048] f32, w [2048, 2048] f32, b [2048] f32 -> [8192, 2048] f32.

Sharding: data-parallel over rows. Each of the 8 NeuronCores gets 1024 rows of
`inputs`, plus a full replica of `w` and `b`; no collectives. Outputs are
concatenated on the host.

Default per-core kernel (_build_nc_v3, k8=6; ~146 us on 8 cores, rms rel err
1.64e-2 vs f64 -- gate is 2e-2):
  - x is cast to bf16 (ACT) and transposed on the PE (bf16 identity-matmul
    transposes, 2 batches of 8 per m-tile through 2 PSUM banks, DVE
    eviction) into resident k-major tiles xt[m] [128, 16k, 128m] bf16.
    bf16 quantization of x costs 1.66e-3 rms.
  - w streams as column halves [128, 1024] f32; ACT Sign emits sign(w)
    directly (exact in bf16/fp8). First 10 k-tiles -> bf16 tiles; last
    k8=6 k-tiles -> fp8e4 pair tiles [128, pair, 2, 1024].
  - hi8[m] = fp8e4(xt[m] fp8 k-range) via ACT copy, emitted INSIDE
    transpose_mtile after the xt writers (emitting a read before its
    writer breaks tile dep tracking -> silent garbage).
  - matmuls: 6 PSUM groups (m, quarter) in flight; per group 10 bf16
    matmuls [128k x 512n] at 1 col/cycle plus 3 fp8 DoubleRow matmuls
    [256k x 512n] at the same 213 ns (2x K per instruction = the only
    2x-rate path on TRN2; bf16 hi+lo would cost the same PE time as f32r).
    PE busy ~105 us = 89 us matmul + 17 us transposes.
  - schedule: SP issues the pure input-DMA stream (x m-tiles 1:2 with w
    half-0, then half-1; ~400 GB/s sustained); block 0 chases the streams
    k-major; later blocks prefetch the next block's transposes at step 2;
    the final (full) block drains with per-group chains so evictions
    stagger. y: DVE bias-add then Pool-engine SWDGE writes.

Error dial: k8 in {0,2,4,6} -> rms err {1.7e-3, 9.6e-3, 1.34e-2, 1.64e-2},
PE matmul time {118, 111, 103, 96} us. k8=6 keeps 18% margin to the 2e-2
gate on the fixed harness inputs (verified exactly via test.py).
"""

import numpy as np

import concourse.bass as bass
import concourse.mybir as mybir
import concourse.tile as tile
from concourse.bass_utils import run_bass_kernel_spmd
from concourse.masks import make_identity

N_CORES = 8
N_ROWS, D_IN, D_OUT = 8192, 2048, 2048
ROWS = N_ROWS // N_CORES  # rows per core
P = 128
K_TILES = D_IN // P  # 16
M_TILES = ROWS // P  # 8
NF = 512  # matmul moving free dim / psum bank width (fp32)
N_TILES = D_OUT // NF  # 4
GROUPS_IN_FLIGHT = 6  # concurrent psum accumulation groups (banks)

F32 = mybir.dt.float32
F32R = mybir.dt.float32r


def _split_waits_pass(nc, max_waits=1, zero_types=("InstDmaTransposeAnt",)):
    """Cap semaphore waits per instruction for this container's walrus.

    The pinned walrus errors ("Too many sync wait commands") when an
    instruction carries more than ~2 sync waits, and errors on ANY wait
    attached to the DIRECT2D_XPOSE struct. Move overflow waits onto
    same-engine NoOps inserted immediately before the instruction; the engine
    executes its stream in order, so the gating semantics are identical.
    """
    idx = 0
    for f in nc.m.functions:
        for bb in f.blocks:
            insts = list(bb.instructions)
            changed = False
            out = []
            for inst in insts:
                si = inst.sync_info
                limit = 0 if type(inst).__name__ in zero_types else max_waits
                if si is not None and si.on_wait and len(si.on_wait) > limit:
                    waits = list(si.on_wait)
                    keep, rest = waits[:limit], waits[limit:]
                    for i in range(0, len(rest), max_waits):
                        nop = mybir.InstNoOp(
                            name=f"splitw-{idx}",
                            ins=[],
                            outs=[],
                            engine=inst.engine,
                            sync_info=mybir.SyncInfo(
                                on_wait=rest[i : i + max_waits], on_update=[]
                            ),
                        )
                        idx += 1
                        out.append(nop)
                    inst.sync_info = mybir.SyncInfo(
                        on_wait=keep, on_update=list(si.on_update or [])
                    )
                    changed = True
                out.append(inst)
            if changed:
                bb.instructions.clear()
                bb.instructions.extend(out)


def _build_nc_f32r():
    nc = bass.Bass()
    x_d = nc.dram_tensor("xs", [ROWS, D_IN], F32, kind="ExternalInput")
    w_d = nc.dram_tensor("w", [D_IN, D_OUT], F32, kind="ExternalInput")
    b_d = nc.dram_tensor("b", [D_OUT], F32, kind="ExternalInput")
    y_d = nc.dram_tensor("y", [ROWS, D_OUT], F32, kind="ExternalOutput")

    with tile.TileContext(nc) as tc:
        with (
            tc.tile_pool(name="const", bufs=1) as const,
            tc.tile_pool(name="s", bufs=2 * K_TILES) as s_pool,
            tc.tile_pool(name="wstage", bufs=6) as wstage,
            tc.tile_pool(name="xstage", bufs=2) as xstage,
            tc.tile_pool(name="xt", bufs=3) as xt_pool,
            tc.tile_pool(name="y", bufs=3) as y_pool,
            tc.tile_pool(name="pst", bufs=2, space="PSUM") as psum_t,
            tc.tile_pool(name="psy", bufs=GROUPS_IN_FLIGHT, space="PSUM") as psum_y,
        ):
            ident = const.tile([P, P], F32)
            make_identity(nc, ident)

            # x tiles: DMA natural layout, transpose 128x128 blocks on PE,
            # evict PSUM->SBUF on DVE
            xts = {}

            def ensure_xt(m):
                if m in xts:
                    return
                xa = xstage.tile([P, D_IN], F32, tag="xstage")
                nc.sync.dma_start(xa[:], x_d[m * P : (m + 1) * P, :])
                xt = xt_pool.tile([P, K_TILES, P], F32R, tag="xt")
                for k in range(K_TILES):
                    pt = psum_t.tile([P, P], F32)
                    nc.tensor.transpose(pt[:], xa[:, k * P : (k + 1) * P], ident[:])
                    nc.vector.tensor_copy(xt[:, k, :], pt[:])
                xts[m] = xt

            # front-load the first x tiles so the PE has transpose work (and
            # block0's matmuls can chase the w chunks) during the w load;
            # m2's DMA must queue ahead of the w stream or block1 stalls on it
            for m in range(3):
                ensure_xt(m)

            # sign(w): resident, loaded as half-width chunks so the DMA
            # issue pipeline is not gated on the sign of the chunk two back
            # (wstage bufs=4 at half size = same SBUF, twice the depth)
            HALF = D_OUT // 2
            s_half = {}
            for k in range(K_TILES):
                for h in range(2):
                    wt = wstage.tile([P, HALF], F32, name=f"wt{k}_{h}", tag="wstage")
                    nc.sync.dma_start(
                        wt[:], w_d[k * P : (k + 1) * P, h * HALF : (h + 1) * HALF]
                    )
                    st = s_pool.tile([P, HALF], F32R, name=f"s{k}_{h}", tag="s")
                    nc.scalar.activation(
                        st[:], wt[:], mybir.ActivationFunctionType.Sign
                    )
                    s_half[(k, h)] = st

            # bias: replicate across all 128 partitions via stride-0 DMA
            # (emitted after the x/w loads; first needed ~35us in)
            b_bcast = const.tile([P, D_OUT], F32)
            nc.sync.dma_start(b_bcast[:], b_d[None, :].to_broadcast([P, D_OUT]))

            groups = [(m, n) for m in range(M_TILES) for n in range(N_TILES)]
            for b0 in range(0, len(groups), GROUPS_IN_FLIGHT):
                block = groups[b0 : b0 + GROUPS_IN_FLIGHT]
                for m, _ in block:
                    ensure_xt(m)
                psums = {}
                for m, n in block:
                    psums[(m, n)] = psum_y.tile(
                        [P, NF], F32, name=f"psum_{m}_{n}", tag="psy"
                    )
                for k in range(K_TILES):
                    for m, n in block:
                        nc.tensor.matmul(
                            psums[(m, n)][:],
                            xts[m][:, k, :],
                            s_half[(k, n // 2)][:, (n % 2) * NF : (n % 2 + 1) * NF],
                            start=(k == 0),
                            stop=(k == K_TILES - 1),
                        )
                    if k == 5:
                        # prefetch the next block's x transposes into this
                        # sweep: during the w-load phase the PE is chunk-
                        # starved here, so the transposes fill the stalls
                        # instead of serializing between sweeps
                        for m, _ in groups[b0 + GROUPS_IN_FLIGHT : b0 + 2 * GROUPS_IN_FLIGHT]:
                            ensure_xt(m)
                for m, n in block:
                    yt = y_pool.tile([P, NF], F32)
                    nc.vector.tensor_add(
                        yt[:], psums[(m, n)][:], b_bcast[:, n * NF : (n + 1) * NF]
                    )
                    nc.sync.dma_start(
                        y_d[m * P : (m + 1) * P, n * NF : (n + 1) * NF], yt[:]
                    )

    _split_waits_pass(nc, max_waits=1)
    return nc



FP8 = mybir.dt.float8e4
K_PAIRS = K_TILES // 2  # 8
NHALF = D_OUT // 2  # 1024


def _build_nc_fp8():
    """fp8 DoubleRow kernel: y = x @ sign(w) + b with x = hi + lo (both fp8e4).

    sign(w) is exactly representable in fp8e4, and splitting x into an fp8
    high part plus an fp8 residual keeps the total quantization error ~7e-4
    relative while running the PE at 2x rate (DoubleRow: K=256 per matmul,
    0.5 cyc/row). w is loaded column-half by column-half so the first
    accumulation groups (n in the left half) only gate on 8 MB of w.
    """
    nc = bass.Bass()
    x_d = nc.dram_tensor("xs", [ROWS, D_IN], F32, kind="ExternalInput")
    w_d = nc.dram_tensor("w", [D_IN, D_OUT], F32, kind="ExternalInput")
    b_d = nc.dram_tensor("b", [D_OUT], F32, kind="ExternalInput")
    y_d = nc.dram_tensor("y", [ROWS, D_OUT], F32, kind="ExternalOutput")

    with tile.TileContext(nc) as tc:
        with (
            tc.tile_pool(name="const", bufs=1) as const,
            tc.tile_pool(name="s8", bufs=2 * K_PAIRS) as s_pool,
            tc.tile_pool(name="wstage", bufs=4) as wstage,
            tc.tile_pool(name="xstage", bufs=2) as xstage,
            tc.tile_pool(name="histage", bufs=2) as histage,
            tc.tile_pool(name="lostage", bufs=2) as lostage,
            tc.tile_pool(name="xt8", bufs=2 * M_TILES) as xt_pool,
            tc.tile_pool(name="y", bufs=3) as y_pool,
            tc.tile_pool(name="pst", bufs=2, space="PSUM") as psum_t,
            tc.tile_pool(name="psy", bufs=GROUPS_IN_FLIGHT, space="PSUM") as psum_y,
        ):
            ident8 = const.tile([P, P], FP8)
            make_identity(nc, ident8)
            b_bcast = const.tile([P, D_OUT], F32)
            nc.sync.dma_start(b_bcast[:], b_d[None, :].to_broadcast([P, D_OUT]))

            # x pipeline: load rows, split into fp8 hi + fp8 residual lo,
            # transpose both 128x128-blockwise on the PE (4 blocks per PSUM
            # tile, strided step-2 as the fp8 transpose requires), evict to
            # resident k-major tiles.
            xt_hi = {}
            xt_lo = {}

            def xpipe(m):
                xa = xstage.tile([P, D_IN], F32, name=f"xa{m}", tag="xa")
                nc.sync.dma_start(xa[:], x_d[m * P : (m + 1) * P, :])
                hi8 = histage.tile([P, D_IN], FP8, name=f"hi{m}", tag="hi")
                nc.scalar.copy(hi8[:], xa[:])
                lo8 = lostage.tile([P, D_IN], FP8, name=f"lo{m}", tag="lo")
                nc.vector.tensor_tensor(
                    lo8[:], xa[:], hi8[:], mybir.AluOpType.subtract
                )
                th = xt_pool.tile([P, K_TILES, P], FP8, name=f"xth{m}", tag="xt")
                tl = xt_pool.tile([P, K_TILES, P], FP8, name=f"xtl{m}", tag="xt")
                for src8, dst in ((hi8, th), (lo8, tl)):
                    for q in range(K_TILES // 4):
                        pt = psum_t.tile([P, 4, P, 2], FP8, name=f"pt{m}", tag="pt")
                        for i in range(4):
                            k = 4 * q + i
                            nc.tensor.transpose(
                                pt[:, i, :, 0],
                                src8[:, k * P : (k + 1) * P],
                                ident8[:],
                            )
                        nc.vector.tensor_copy(dst[:, 4 * q : 4 * q + 4, :], pt[:, :, :, 0])
                xt_hi[m] = th
                xt_lo[m] = tl

            # sign(w) per column half: s8[(pair, half)] = [P, 2, NHALF] fp8
            s8 = {}

            def load_w_half(h):
                for k in range(K_TILES):
                    wt = wstage.tile([P, NHALF], F32, name=f"w{h}_{k}", tag="w")
                    nc.sync.dma_start(
                        wt[:], w_d[k * P : (k + 1) * P, h * NHALF : (h + 1) * NHALF]
                    )
                    j = k // 2
                    if (j, h) not in s8:
                        s8[(j, h)] = s_pool.tile(
                            [P, 2, NHALF], FP8, name=f"s{j}_{h}", tag="s"
                        )
                    nc.scalar.activation(
                        s8[(j, h)][:, k % 2, :],
                        wt[:],
                        mybir.ActivationFunctionType.Sign,
                    )

            for m in range(3):
                xpipe(m)
            load_w_half(0)
            for m in range(3, M_TILES):
                xpipe(m)
            load_w_half(1)

            # accumulation groups, left column half first
            groups = [(m, n) for h in range(2) for m in range(M_TILES) for n in (2 * h, 2 * h + 1)]
            for b0 in range(0, len(groups), GROUPS_IN_FLIGHT):
                block = groups[b0 : b0 + GROUPS_IN_FLIGHT]
                psums = {}
                for m, n in block:
                    psums[(m, n)] = psum_y.tile(
                        [P, NF], F32, name=f"psum_{m}_{n}", tag="psy"
                    )
                for pi, xt in enumerate((xt_hi, xt_lo)):
                    for j in range(K_PAIRS):
                        for m, n in block:
                            nc.tensor.matmul(
                                psums[(m, n)][:],
                                xt[m][:, 2 * j : 2 * j + 2, :],
                                s8[(j, n // 2)][:, :, (n % 2) * NF : (n % 2 + 1) * NF],
                                perf_mode=mybir.MatmulPerfMode.DoubleRow,
                                start=(pi == 0 and j == 0),
                                stop=(pi == 1 and j == K_PAIRS - 1),
                            )
                for m, n in block:
                    yt = y_pool.tile([P, NF], F32)
                    nc.vector.tensor_add(
                        yt[:], psums[(m, n)][:], b_bcast[:, n * NF : (n + 1) * NF]
                    )
                    nc.sync.dma_start(
                        y_d[m * P : (m + 1) * P, n * NF : (n + 1) * NF], yt[:]
                    )

    _split_waits_pass(nc, max_waits=1)
    return nc


BF16 = mybir.dt.bfloat16
U16 = mybir.dt.uint16


def _build_nc_v3(k8=0):
    """bf16/fp8 kernel on the proven f32r skeleton (PE transposes, no xbar).

    - x m-tiles: DMA f32 (sync, interleaved with w) -> ACT cast to bf16 ->
      PE transpose in 2 batches of 8 into a PSUM bank -> DVE evict to
      resident k-major bf16 tiles xt[m] [128, 16, 128].
    - w streams as halves [128, 1024] f32; ACT Sign -> resident bf16 tiles
      (and fp8e4 pair tiles for the last k8 k-tiles).
    - matmuls: 6 PSUM groups (m, quarter) in flight (2 banks reserved for
      transposes), k-major in-block; per group kbf bf16 matmuls + kp8 fp8
      DoubleRow matmuls (2x rate).
    - evict: DVE bias-add -> SBUF, y written per (m, quarter) via Pool SWDGE.

    k8 = k-tiles (of 16) in fp8 single-pass; adds ~2.7e-2*sqrt(k8/16) error.
    """
    assert k8 % 2 == 0
    kbf = K_TILES - k8
    kp8 = k8 // 2
    NH = 1024
    GROUPS = 6

    nc = bass.Bass()
    x_d = nc.dram_tensor("xs", [ROWS, D_IN], F32, kind="ExternalInput")
    w_d = nc.dram_tensor("w", [D_IN, D_OUT], F32, kind="ExternalInput")
    b_d = nc.dram_tensor("b", [D_OUT], F32, kind="ExternalInput")
    y_d = nc.dram_tensor("y", [ROWS, D_OUT], F32, kind="ExternalOutput")

    with tile.TileContext(nc) as tc:
        with (
            tc.tile_pool(name="const", bufs=1) as const,
            tc.tile_pool(name="xstage", bufs=3) as xstage,
            tc.tile_pool(name="xbstage", bufs=3) as xbstage,
            tc.tile_pool(name="xt", bufs=M_TILES) as xt_pool,
            tc.tile_pool(name="hi8", bufs=M_TILES) as hi8_pool,
            tc.tile_pool(name="s", bufs=2) as s_pool,
            tc.tile_pool(name="s8", bufs=2) as s8_pool,
            tc.tile_pool(name="wstage", bufs=6) as wstage,
            tc.tile_pool(name="y", bufs=4) as y_pool,
            tc.tile_pool(name="ident", bufs=1) as ident_pool,
            tc.tile_pool(name="pst", bufs=2, space="PSUM") as psum_t,
            tc.tile_pool(name="psy", bufs=GROUPS, space="PSUM") as psum_y,
        ):
            ident = ident_pool.tile([P, P], BF16)
            make_identity(nc, ident)
            b_bcast = const.tile([P, D_OUT], F32)

            # tiles
            xas = [xstage.tile([P, D_IN], F32, name=f"xa{m}", tag="xa") for m in range(M_TILES)]
            xbs = [xbstage.tile([P, D_IN], BF16, name=f"xb{m}", tag="xb") for m in range(M_TILES)]
            xts = [xt_pool.tile([P, K_TILES, P], BF16, name=f"xt{m}", tag="xt") for m in range(M_TILES)]
            s_bfs = [
                s_pool.tile([P, kbf, NH], BF16, name=f"s{h}", tag="s") if kbf else None
                for h in range(2)
            ]
            s8s = [
                s8_pool.tile([P, kp8, 2, NH], FP8, name=f"s8{h}", tag="s8") if kp8 else None
                for h in range(2)
            ]
            hi8s = [
                hi8_pool.tile([P, k8, P], FP8, name=f"hi{m}", tag="hi") if kp8 else None
                for m in range(M_TILES)
            ]
            wts = {}

            # --- SP: pure input DMA stream (x 1:2 with w half-0, then half-1)
            def dma_w(h, k):
                wt = wstage.tile([P, NH], F32, name=f"w{h}_{k}", tag="w")
                nc.sync.dma_start(wt[:], w_d[k * P : (k + 1) * P, h * NH : (h + 1) * NH])
                wts[(h, k)] = wt

            for m in range(M_TILES):
                nc.sync.dma_start(xas[m][:], x_d[m * P : (m + 1) * P, :])
                dma_w(0, 2 * m)
                dma_w(0, 2 * m + 1)
                if m == 2:
                    nc.sync.dma_start(b_bcast[:], b_d[None, :].to_broadcast([P, D_OUT]))
            for k in range(K_TILES):
                dma_w(1, k)

            # --- ACT: casts + signs woven in need-order; hi8 fp8 casts last
            def emit_sign(h, k):
                if k < kbf:
                    nc.scalar.activation(
                        s_bfs[h][:, k, :], wts[(h, k)][:],
                        mybir.ActivationFunctionType.Sign,
                    )
                else:
                    j, par = divmod(k - kbf, 2)
                    nc.scalar.activation(
                        s8s[h][:, j, par, :], wts[(h, k)][:],
                        mybir.ActivationFunctionType.Sign,
                    )

            for m in range(M_TILES):
                nc.scalar.copy(xbs[m][:], xas[m][:])
                emit_sign(0, 2 * m)
                emit_sign(0, 2 * m + 1)
            # h1 signs are emitted lazily (before the first block that reads
            # them) so the ACT stream serves hi8 copies first; hi8 copies are
            # emitted inside transpose_mtile AFTER the xt writers exist --
            # emitting a read before its writer breaks dep tracking.
            h1_signed = [False]

            def ensure_h1_signs():
                if h1_signed[0]:
                    return
                h1_signed[0] = True
                for k in range(K_TILES):
                    emit_sign(1, k)

            # --- PE: transposes (2 batches of 8 per m-tile) woven with
            # matmuls; DVE evictions; Pool y writes.
            transposed = [False] * M_TILES

            def transpose_mtile(m):
                if transposed[m]:
                    return
                transposed[m] = True
                for half in range(2):
                    pt = psum_t.tile([P, 8, P], BF16, name=f"pt{m}_{half}", tag="pt")
                    for i in range(8):
                        k = 8 * half + i
                        nc.tensor.transpose(
                            pt[:, i, :], xbs[m][:, k * P : (k + 1) * P], ident[:]
                        )
                    nc.vector.tensor_copy(xts[m][:, 8 * half : 8 * half + 8, :], pt[:])
                if kp8:
                    nc.scalar.copy(hi8s[m][:], xts[m][:, kbf:, :])

            steps = [("bf", k) for k in range(kbf)] + [("f8", j) for j in range(kp8)]

            def emit_mm(ps, m, qq, step, first, last):
                kind, i = step
                h = qq // 2
                sl = slice((qq % 2) * NF, (qq % 2) * NF + NF)
                if kind == "bf":
                    nc.tensor.matmul(
                        ps[:], xts[m][:, i, :], s_bfs[h][:, i, sl],
                        start=first, stop=last,
                    )
                else:
                    nc.tensor.matmul(
                        ps[:], hi8s[m][:, 2 * i : 2 * i + 2, :], s8s[h][:, i, :, sl],
                        perf_mode=mybir.MatmulPerfMode.DoubleRow,
                        start=first, stop=last,
                    )

            def evict(ps, m, qq):
                yt = y_pool.tile([P, NF], F32, name=f"yt{qq}_{m}", tag="y")
                nc.vector.tensor_add(yt[:], ps[:], b_bcast[:, qq * NF : (qq + 1) * NF])
                nc.gpsimd.dma_start(
                    y_d[m * P : (m + 1) * P, qq * NF : (qq + 1) * NF], yt[:]
                )

            # front-load first 3 m-tile transposes (they gate block 0)
            for m in range(3):
                transpose_mtile(m)

            groups = [(m, qq) for qq in range(4) for m in range(M_TILES)]
            # short block first: block 0 is input-bound regardless, and this
            # makes the final block a full one whose per-group drain staggers
            rem = len(groups) % GROUPS
            bounds = [0, rem] if rem else [0]
            while bounds[-1] < len(groups):
                bounds.append(bounds[-1] + GROUPS)
            blocks_list = [groups[a:b] for a, b in zip(bounds, bounds[1:])]
            first_h1 = next(
                i for i, blk in enumerate(blocks_list) if any(q >= 2 for _, q in blk)
            )
            n_blocks = len(blocks_list)
            for bi in range(n_blocks):
                block = blocks_list[bi]
                nxt = blocks_list[bi + 1] if bi + 1 < n_blocks else []
                for m, _ in block:
                    transpose_mtile(m)
                if bi + 1 >= first_h1:
                    ensure_h1_signs()
                psums = {
                    (m, qq): psum_y.tile([P, NF], F32, name=f"ps{qq}_{m}", tag="psy")
                    for m, qq in block
                }
                if bi == n_blocks - 1:
                    # drain block: per-group chains so evictions start early
                    for m, qq in block:
                        for si, st in enumerate(steps):
                            emit_mm(psums[(m, qq)], m, qq, st, si == 0, si == len(steps) - 1)
                        evict(psums[(m, qq)], m, qq)
                    continue
                for si, st in enumerate(steps):
                    for m, qq in block:
                        emit_mm(psums[(m, qq)], m, qq, st, si == 0, si == len(steps) - 1)
                    if si == 2:
                        # early prefetch: next block's transposes (and their
                        # hi8 copies) fill the PE's w-starved stalls
                        for m, _ in nxt:
                            transpose_mtile(m)
                for m, qq in block:
                    evict(psums[(m, qq)], m, qq)

            ensure_h1_signs()
            if __import__("os").environ.get("BASS_V3_DEBUG"):
                xt_dump = nc.dram_tensor("xt_dump", [M_TILES, P, K_TILES, P], BF16, kind="ExternalOutput")
                for m in range(M_TILES):
                    nc.sync.dma_start(xt_dump[m], xts[m][:])
                if kp8:
                    hi_dump = nc.dram_tensor("hi_dump", [M_TILES, P, k8, P], FP8, kind="ExternalOutput")
                    s8_dump = nc.dram_tensor("s8_dump", [2, P, kp8, 2, NH], FP8, kind="ExternalOutput")
                    for m in range(M_TILES):
                        nc.sync.dma_start(hi_dump[m], hi8s[m][:])
                    for h in range(2):
                        nc.sync.dma_start(s8_dump[h], s8s[h][:])

    _split_waits_pass(nc, max_waits=1)
    return nc


def _build_nc():
    import os

    impl = os.environ.get("BASS_DENSE_IMPL", "v3")
    if impl == "f32r":
        return _build_nc_f32r()
    if impl == "fp8":
        return _build_nc_fp8()
    k8 = int(os.environ.get("BASS_K8", "6"))
    return _build_nc_v3(k8=k8)


_NC_CACHE = None


def _get_nc():
    global _NC_CACHE
    if _NC_CACHE is None:
        _NC_CACHE = _build_nc()
    return _NC_CACHE


def _run(inputs, w, b, trace=False):
    nc = _get_nc()
    inputs = np.ascontiguousarray(inputs, dtype=np.float32)
    w = np.ascontiguousarray(w, dtype=np.float32)
    b = np.ascontiguousarray(b, dtype=np.float32)
    in_maps = [
        {"xs": np.ascontiguousarray(inputs[i * ROWS : (i + 1) * ROWS]), "w": w, "b": b}
        for i in range(N_CORES)
    ]
    res = run_bass_kernel_spmd(nc, in_maps, list(range(N_CORES)), trace=trace)
    out = np.concatenate([res.results[i]["y"] for i in range(N_CORES)], axis=0)
    return out, res


def kernel(inputs, w, b):
    out, _ = _run(inputs, w, b, trace=False)
    return out



# revision 15
# speedup vs baseline: 1.0060x; 1.0060x over previous
"""Trainium2 Bass kernel for nn_DenseTf: out = inputs @ sign(clip(w,-1,1)) + b.

Shapes: inputs [8192, 2048] f32, w [2048, 2048] f32, b [2048] f32 -> [8192, 2048] f32.

Sharding: data-parallel over rows. Each of the 8 NeuronCores gets 1024 rows of
`inputs`, plus a full replica of `w` and `b`; no collectives. Outputs are
concatenated on the host.

Default per-core kernel (_build_nc_v3, k8=6; ~146 us on 8 cores, rms rel err
1.64e-2 vs f64 -- gate is 2e-2):
  - x is cast to bf16 (ACT) and transposed on the PE (bf16 identity-matmul
    transposes, 2 batches of 8 per m-tile through 2 PSUM banks, DVE
    eviction) into resident k-major tiles xt[m] [128, 16k, 128m] bf16.
    bf16 quantization of x costs 1.66e-3 rms.
  - w streams as column halves [128, 1024] f32; ACT Sign emits sign(w)
    directly (exact in bf16/fp8). First 10 k-tiles -> bf16 tiles; last
    k8=6 k-tiles -> fp8e4 pair tiles [128, pair, 2, 1024].
  - hi8[m] = fp8e4(xt[m] fp8 k-range) via ACT copy, emitted INSIDE
    transpose_mtile after the xt writers (emitting a read before its
    writer breaks tile dep tracking -> silent garbage).
  - matmuls: 6 PSUM groups (m, quarter) in flight; per group 10 bf16
    matmuls [128k x 512n] at 1 col/cycle plus 3 fp8 DoubleRow matmuls
    [256k x 512n] at the same 213 ns (2x K per instruction = the only
    2x-rate path on TRN2; bf16 hi+lo would cost the same PE time as f32r).
    PE busy ~105 us = 89 us matmul + 17 us transposes.
  - schedule: SP issues the pure input-DMA stream (x m-tiles 1:2 with w
    half-0, then half-1; ~400 GB/s sustained); block 0 chases the streams
    k-major; later blocks prefetch the next block's transposes at step 2;
    the final (full) block drains with per-group chains so evictions
    stagger. y: DVE bias-add then Pool-engine SWDGE writes.

Error dial: k8 in {0,2,4,6} -> rms err {1.7e-3, 9.6e-3, 1.34e-2, 1.64e-2},
PE matmul time {118, 111, 103, 96} us. k8=6 keeps 18% margin to the 2e-2
gate on the fixed harness inputs (verified exactly via test.py).
"""

import numpy as np

import concourse.bass as bass
import concourse.mybir as mybir
import concourse.tile as tile
from concourse.bass_utils import run_bass_kernel_spmd
from concourse.masks import make_identity

N_CORES = 8
N_ROWS, D_IN, D_OUT = 8192, 2048, 2048
ROWS = N_ROWS // N_CORES  # rows per core
P = 128
K_TILES = D_IN // P  # 16
M_TILES = ROWS // P  # 8
NF = 512  # matmul moving free dim / psum bank width (fp32)
N_TILES = D_OUT // NF  # 4
GROUPS_IN_FLIGHT = 6  # concurrent psum accumulation groups (banks)

F32 = mybir.dt.float32
F32R = mybir.dt.float32r


def _split_waits_pass(nc, max_waits=1, zero_types=("InstDmaTransposeAnt",)):
    """Cap semaphore waits per instruction for this container's walrus.

    The pinned walrus errors ("Too many sync wait commands") when an
    instruction carries more than ~2 sync waits, and errors on ANY wait
    attached to the DIRECT2D_XPOSE struct. Move overflow waits onto
    same-engine NoOps inserted immediately before the instruction; the engine
    executes its stream in order, so the gating semantics are identical.
    """
    idx = 0
    for f in nc.m.functions:
        for bb in f.blocks:
            insts = list(bb.instructions)
            changed = False
            out = []
            for inst in insts:
                si = inst.sync_info
                limit = 0 if type(inst).__name__ in zero_types else max_waits
                if si is not None and si.on_wait and len(si.on_wait) > limit:
                    waits = list(si.on_wait)
                    keep, rest = waits[:limit], waits[limit:]
                    for i in range(0, len(rest), max_waits):
                        nop = mybir.InstNoOp(
                            name=f"splitw-{idx}",
                            ins=[],
                            outs=[],
                            engine=inst.engine,
                            sync_info=mybir.SyncInfo(
                                on_wait=rest[i : i + max_waits], on_update=[]
                            ),
                        )
                        idx += 1
                        out.append(nop)
                    inst.sync_info = mybir.SyncInfo(
                        on_wait=keep, on_update=list(si.on_update or [])
                    )
                    changed = True
                out.append(inst)
            if changed:
                bb.instructions.clear()
                bb.instructions.extend(out)


def _build_nc_f32r():
    nc = bass.Bass()
    x_d = nc.dram_tensor("xs", [ROWS, D_IN], F32, kind="ExternalInput")
    w_d = nc.dram_tensor("w", [D_IN, D_OUT], F32, kind="ExternalInput")
    b_d = nc.dram_tensor("b", [D_OUT], F32, kind="ExternalInput")
    y_d = nc.dram_tensor("y", [ROWS, D_OUT], F32, kind="ExternalOutput")

    with tile.TileContext(nc) as tc:
        with (
            tc.tile_pool(name="const", bufs=1) as const,
            tc.tile_pool(name="s", bufs=2 * K_TILES) as s_pool,
            tc.tile_pool(name="wstage", bufs=6) as wstage,
            tc.tile_pool(name="xstage", bufs=2) as xstage,
            tc.tile_pool(name="xt", bufs=3) as xt_pool,
            tc.tile_pool(name="y", bufs=3) as y_pool,
            tc.tile_pool(name="pst", bufs=2, space="PSUM") as psum_t,
            tc.tile_pool(name="psy", bufs=GROUPS_IN_FLIGHT, space="PSUM") as psum_y,
        ):
            ident = const.tile([P, P], F32)
            make_identity(nc, ident)

            # x tiles: DMA natural layout, transpose 128x128 blocks on PE,
            # evict PSUM->SBUF on DVE
            xts = {}

            def ensure_xt(m):
                if m in xts:
                    return
                xa = xstage.tile([P, D_IN], F32, tag="xstage")
                nc.sync.dma_start(xa[:], x_d[m * P : (m + 1) * P, :])
                xt = xt_pool.tile([P, K_TILES, P], F32R, tag="xt")
                for k in range(K_TILES):
                    pt = psum_t.tile([P, P], F32)
                    nc.tensor.transpose(pt[:], xa[:, k * P : (k + 1) * P], ident[:])
                    nc.vector.tensor_copy(xt[:, k, :], pt[:])
                xts[m] = xt

            # front-load the first x tiles so the PE has transpose work (and
            # block0's matmuls can chase the w chunks) during the w load;
            # m2's DMA must queue ahead of the w stream or block1 stalls on it
            for m in range(3):
                ensure_xt(m)

            # sign(w): resident, loaded as half-width chunks so the DMA
            # issue pipeline is not gated on the sign of the chunk two back
            # (wstage bufs=4 at half size = same SBUF, twice the depth)
            HALF = D_OUT // 2
            s_half = {}
            for k in range(K_TILES):
                for h in range(2):
                    wt = wstage.tile([P, HALF], F32, name=f"wt{k}_{h}", tag="wstage")
                    nc.sync.dma_start(
                        wt[:], w_d[k * P : (k + 1) * P, h * HALF : (h + 1) * HALF]
                    )
                    st = s_pool.tile([P, HALF], F32R, name=f"s{k}_{h}", tag="s")
                    nc.scalar.activation(
                        st[:], wt[:], mybir.ActivationFunctionType.Sign
                    )
                    s_half[(k, h)] = st

            # bias: replicate across all 128 partitions via stride-0 DMA
            # (emitted after the x/w loads; first needed ~35us in)
            b_bcast = const.tile([P, D_OUT], F32)
            nc.sync.dma_start(b_bcast[:], b_d[None, :].to_broadcast([P, D_OUT]))

            groups = [(m, n) for m in range(M_TILES) for n in range(N_TILES)]
            for b0 in range(0, len(groups), GROUPS_IN_FLIGHT):
                block = groups[b0 : b0 + GROUPS_IN_FLIGHT]
                for m, _ in block:
                    ensure_xt(m)
                psums = {}
                for m, n in block:
                    psums[(m, n)] = psum_y.tile(
                        [P, NF], F32, name=f"psum_{m}_{n}", tag="psy"
                    )
                for k in range(K_TILES):
                    for m, n in block:
                        nc.tensor.matmul(
                            psums[(m, n)][:],
                            xts[m][:, k, :],
                            s_half[(k, n // 2)][:, (n % 2) * NF : (n % 2 + 1) * NF],
                            start=(k == 0),
                            stop=(k == K_TILES - 1),
                        )
                    if k == 5:
                        # prefetch the next block's x transposes into this
                        # sweep: during the w-load phase the PE is chunk-
                        # starved here, so the transposes fill the stalls
                        # instead of serializing between sweeps
                        for m, _ in groups[b0 + GROUPS_IN_FLIGHT : b0 + 2 * GROUPS_IN_FLIGHT]:
                            ensure_xt(m)
                for m, n in block:
                    yt = y_pool.tile([P, NF], F32)
                    nc.vector.tensor_add(
                        yt[:], psums[(m, n)][:], b_bcast[:, n * NF : (n + 1) * NF]
                    )
                    nc.sync.dma_start(
                        y_d[m * P : (m + 1) * P, n * NF : (n + 1) * NF], yt[:]
                    )

    _split_waits_pass(nc, max_waits=1)
    return nc



FP8 = mybir.dt.float8e4
K_PAIRS = K_TILES // 2  # 8
NHALF = D_OUT // 2  # 1024


def _build_nc_fp8():
    """fp8 DoubleRow kernel: y = x @ sign(w) + b with x = hi + lo (both fp8e4).

    sign(w) is exactly representable in fp8e4, and splitting x into an fp8
    high part plus an fp8 residual keeps the total quantization error ~7e-4
    relative while running the PE at 2x rate (DoubleRow: K=256 per matmul,
    0.5 cyc/row). w is loaded column-half by column-half so the first
    accumulation groups (n in the left half) only gate on 8 MB of w.
    """
    nc = bass.Bass()
    x_d = nc.dram_tensor("xs", [ROWS, D_IN], F32, kind="ExternalInput")
    w_d = nc.dram_tensor("w", [D_IN, D_OUT], F32, kind="ExternalInput")
    b_d = nc.dram_tensor("b", [D_OUT], F32, kind="ExternalInput")
    y_d = nc.dram_tensor("y", [ROWS, D_OUT], F32, kind="ExternalOutput")

    with tile.TileContext(nc) as tc:
        with (
            tc.tile_pool(name="const", bufs=1) as const,
            tc.tile_pool(name="s8", bufs=2 * K_PAIRS) as s_pool,
            tc.tile_pool(name="wstage", bufs=4) as wstage,
            tc.tile_pool(name="xstage", bufs=2) as xstage,
            tc.tile_pool(name="histage", bufs=2) as histage,
            tc.tile_pool(name="lostage", bufs=2) as lostage,
            tc.tile_pool(name="xt8", bufs=2 * M_TILES) as xt_pool,
            tc.tile_pool(name="y", bufs=3) as y_pool,
            tc.tile_pool(name="pst", bufs=2, space="PSUM") as psum_t,
            tc.tile_pool(name="psy", bufs=GROUPS_IN_FLIGHT, space="PSUM") as psum_y,
        ):
            ident8 = const.tile([P, P], FP8)
            make_identity(nc, ident8)
            b_bcast = const.tile([P, D_OUT], F32)
            nc.sync.dma_start(b_bcast[:], b_d[None, :].to_broadcast([P, D_OUT]))

            # x pipeline: load rows, split into fp8 hi + fp8 residual lo,
            # transpose both 128x128-blockwise on the PE (4 blocks per PSUM
            # tile, strided step-2 as the fp8 transpose requires), evict to
            # resident k-major tiles.
            xt_hi = {}
            xt_lo = {}

            def xpipe(m):
                xa = xstage.tile([P, D_IN], F32, name=f"xa{m}", tag="xa")
                nc.sync.dma_start(xa[:], x_d[m * P : (m + 1) * P, :])
                hi8 = histage.tile([P, D_IN], FP8, name=f"hi{m}", tag="hi")
                nc.scalar.copy(hi8[:], xa[:])
                lo8 = lostage.tile([P, D_IN], FP8, name=f"lo{m}", tag="lo")
                nc.vector.tensor_tensor(
                    lo8[:], xa[:], hi8[:], mybir.AluOpType.subtract
                )
                th = xt_pool.tile([P, K_TILES, P], FP8, name=f"xth{m}", tag="xt")
                tl = xt_pool.tile([P, K_TILES, P], FP8, name=f"xtl{m}", tag="xt")
                for src8, dst in ((hi8, th), (lo8, tl)):
                    for q in range(K_TILES // 4):
                        pt = psum_t.tile([P, 4, P, 2], FP8, name=f"pt{m}", tag="pt")
                        for i in range(4):
                            k = 4 * q + i
                            nc.tensor.transpose(
                                pt[:, i, :, 0],
                                src8[:, k * P : (k + 1) * P],
                                ident8[:],
                            )
                        nc.vector.tensor_copy(dst[:, 4 * q : 4 * q + 4, :], pt[:, :, :, 0])
                xt_hi[m] = th
                xt_lo[m] = tl

            # sign(w) per column half: s8[(pair, half)] = [P, 2, NHALF] fp8
            s8 = {}

            def load_w_half(h):
                for k in range(K_TILES):
                    wt = wstage.tile([P, NHALF], F32, name=f"w{h}_{k}", tag="w")
                    nc.sync.dma_start(
                        wt[:], w_d[k * P : (k + 1) * P, h * NHALF : (h + 1) * NHALF]
                    )
                    j = k // 2
                    if (j, h) not in s8:
                        s8[(j, h)] = s_pool.tile(
                            [P, 2, NHALF], FP8, name=f"s{j}_{h}", tag="s"
                        )
                    nc.scalar.activation(
                        s8[(j, h)][:, k % 2, :],
                        wt[:],
                        mybir.ActivationFunctionType.Sign,
                    )

            for m in range(3):
                xpipe(m)
            load_w_half(0)
            for m in range(3, M_TILES):
                xpipe(m)
            load_w_half(1)

            # accumulation groups, left column half first
            groups = [(m, n) for h in range(2) for m in range(M_TILES) for n in (2 * h, 2 * h + 1)]
            for b0 in range(0, len(groups), GROUPS_IN_FLIGHT):
                block = groups[b0 : b0 + GROUPS_IN_FLIGHT]
                psums = {}
                for m, n in block:
                    psums[(m, n)] = psum_y.tile(
                        [P, NF], F32, name=f"psum_{m}_{n}", tag="psy"
                    )
                for pi, xt in enumerate((xt_hi, xt_lo)):
                    for j in range(K_PAIRS):
                        for m, n in block:
                            nc.tensor.matmul(
                                psums[(m, n)][:],
                                xt[m][:, 2 * j : 2 * j + 2, :],
                                s8[(j, n // 2)][:, :, (n % 2) * NF : (n % 2 + 1) * NF],
                                perf_mode=mybir.MatmulPerfMode.DoubleRow,
                                start=(pi == 0 and j == 0),
                                stop=(pi == 1 and j == K_PAIRS - 1),
                            )
                for m, n in block:
                    yt = y_pool.tile([P, NF], F32)
                    nc.vector.tensor_add(
                        yt[:], psums[(m, n)][:], b_bcast[:, n * NF : (n + 1) * NF]
                    )
                    nc.sync.dma_start(
                        y_d[m * P : (m + 1) * P, n * NF : (n + 1) * NF], yt[:]
                    )

    _split_waits_pass(nc, max_waits=1)
    return nc


BF16 = mybir.dt.bfloat16
U16 = mybir.dt.uint16


def _build_nc_v3(k8=0):
    """bf16/fp8 kernel on the proven f32r skeleton (PE transposes, no xbar).

    - x m-tiles: DMA f32 (sync, interleaved with w) -> ACT cast to bf16 ->
      PE transpose in 2 batches of 8 into a PSUM bank -> DVE evict to
      resident k-major bf16 tiles xt[m] [128, 16, 128].
    - w streams as halves [128, 1024] f32; ACT Sign -> resident bf16 tiles
      (and fp8e4 pair tiles for the last k8 k-tiles).
    - matmuls: 6 PSUM groups (m, quarter) in flight (2 banks reserved for
      transposes), k-major in-block; per group kbf bf16 matmuls + kp8 fp8
      DoubleRow matmuls (2x rate).
    - evict: DVE bias-add -> SBUF, y written per (m, quarter) via Pool SWDGE.

    k8 = k-tiles (of 16) in fp8 single-pass; adds ~2.7e-2*sqrt(k8/16) error.
    """
    assert k8 % 2 == 0
    kbf = K_TILES - k8
    kp8 = k8 // 2
    NH = 1024
    GROUPS = 6

    nc = bass.Bass()
    x_d = nc.dram_tensor("xs", [ROWS, D_IN], F32, kind="ExternalInput")
    w_d = nc.dram_tensor("w", [D_IN, D_OUT], F32, kind="ExternalInput")
    b_d = nc.dram_tensor("b", [D_OUT], F32, kind="ExternalInput")
    y_d = nc.dram_tensor("y", [ROWS, D_OUT], F32, kind="ExternalOutput")

    with tile.TileContext(nc) as tc:
        with (
            tc.tile_pool(name="const", bufs=1) as const,
            tc.tile_pool(name="xstage", bufs=3) as xstage,
            tc.tile_pool(name="xbstage", bufs=3) as xbstage,
            tc.tile_pool(name="xt", bufs=M_TILES) as xt_pool,
            tc.tile_pool(name="hi8", bufs=M_TILES) as hi8_pool,
            tc.tile_pool(name="s", bufs=2) as s_pool,
            tc.tile_pool(name="s8", bufs=2) as s8_pool,
            tc.tile_pool(name="wstage", bufs=6) as wstage,
            tc.tile_pool(name="y", bufs=4) as y_pool,
            tc.tile_pool(name="ident", bufs=1) as ident_pool,
            tc.tile_pool(name="pst", bufs=2, space="PSUM") as psum_t,
            tc.tile_pool(name="psy", bufs=GROUPS, space="PSUM") as psum_y,
        ):
            ident = ident_pool.tile([P, P], BF16)
            make_identity(nc, ident)
            b_bcast = const.tile([P, D_OUT], F32)

            # tiles
            xas = [xstage.tile([P, D_IN], F32, name=f"xa{m}", tag="xa") for m in range(M_TILES)]
            xbs = [xbstage.tile([P, D_IN], BF16, name=f"xb{m}", tag="xb") for m in range(M_TILES)]
            xts = [xt_pool.tile([P, K_TILES, P], BF16, name=f"xt{m}", tag="xt") for m in range(M_TILES)]
            s_bfs = [
                s_pool.tile([P, kbf, NH], BF16, name=f"s{h}", tag="s") if kbf else None
                for h in range(2)
            ]
            s8s = [
                s8_pool.tile([P, kp8, 2, NH], FP8, name=f"s8{h}", tag="s8") if kp8 else None
                for h in range(2)
            ]
            hi8s = [
                hi8_pool.tile([P, k8, P], FP8, name=f"hi{m}", tag="hi") if kp8 else None
                for m in range(M_TILES)
            ]
            wts = {}

            # --- SP: pure input DMA stream (x 1:2 with w half-0, then half-1)
            def dma_w(h, k):
                wt = wstage.tile([P, NH], F32, name=f"w{h}_{k}", tag="w")
                nc.sync.dma_start(wt[:], w_d[k * P : (k + 1) * P, h * NH : (h + 1) * NH])
                wts[(h, k)] = wt

            for m in range(M_TILES):
                nc.sync.dma_start(xas[m][:], x_d[m * P : (m + 1) * P, :])
                dma_w(0, 2 * m)
                dma_w(0, 2 * m + 1)
                if m == 2:
                    nc.sync.dma_start(b_bcast[:], b_d[None, :].to_broadcast([P, D_OUT]))
            for k in range(K_TILES):
                dma_w(1, k)

            # --- ACT: casts + signs woven in need-order; hi8 fp8 casts last
            def emit_sign(h, k):
                if k < kbf:
                    nc.scalar.activation(
                        s_bfs[h][:, k, :], wts[(h, k)][:],
                        mybir.ActivationFunctionType.Sign,
                    )
                else:
                    j, par = divmod(k - kbf, 2)
                    nc.scalar.activation(
                        s8s[h][:, j, par, :], wts[(h, k)][:],
                        mybir.ActivationFunctionType.Sign,
                    )

            for m in range(M_TILES):
                nc.scalar.copy(xbs[m][:], xas[m][:])
                emit_sign(0, 2 * m)
                emit_sign(0, 2 * m + 1)
            # h1 signs are emitted lazily (before the first block that reads
            # them) so the ACT stream serves hi8 copies first; hi8 copies are
            # emitted inside transpose_mtile AFTER the xt writers exist --
            # emitting a read before its writer breaks dep tracking.
            h1_signed = [False]

            def ensure_h1_signs():
                if h1_signed[0]:
                    return
                h1_signed[0] = True
                for k in range(K_TILES):
                    emit_sign(1, k)

            # --- PE: transposes (2 batches of 8 per m-tile) woven with
            # matmuls; DVE evictions; Pool y writes.
            transposed = [False] * M_TILES

            def transpose_mtile(m):
                if transposed[m]:
                    return
                transposed[m] = True
                for half in range(2):
                    pt = psum_t.tile([P, 8, P], BF16, name=f"pt{m}_{half}", tag="pt")
                    for i in range(8):
                        k = 8 * half + i
                        nc.tensor.transpose(
                            pt[:, i, :], xbs[m][:, k * P : (k + 1) * P], ident[:]
                        )
                    nc.vector.tensor_copy(xts[m][:, 8 * half : 8 * half + 8, :], pt[:])
                if kp8:
                    nc.scalar.copy(hi8s[m][:], xts[m][:, kbf:, :])

            steps = [("bf", k) for k in range(kbf)] + [("f8", j) for j in range(kp8)]

            def emit_mm(ps, m, qq, step, first, last):
                kind, i = step
                h = qq // 2
                sl = slice((qq % 2) * NF, (qq % 2) * NF + NF)
                if kind == "bf":
                    nc.tensor.matmul(
                        ps[:], xts[m][:, i, :], s_bfs[h][:, i, sl],
                        start=first, stop=last,
                    )
                else:
                    nc.tensor.matmul(
                        ps[:], hi8s[m][:, 2 * i : 2 * i + 2, :], s8s[h][:, i, :, sl],
                        perf_mode=mybir.MatmulPerfMode.DoubleRow,
                        start=first, stop=last,
                    )

            def evict(ps, m, qq):
                yt = y_pool.tile([P, NF], F32, name=f"yt{qq}_{m}", tag="y")
                nc.vector.tensor_add(yt[:], ps[:], b_bcast[:, qq * NF : (qq + 1) * NF])
                nc.gpsimd.dma_start(
                    y_d[m * P : (m + 1) * P, qq * NF : (qq + 1) * NF], yt[:]
                )

            # front-load first 3 m-tile transposes (they gate block 0)
            for m in range(3):
                transpose_mtile(m)

            groups = [(m, qq) for qq in range(4) for m in range(M_TILES)]
            n_blocks = (len(groups) + GROUPS - 1) // GROUPS
            for bi in range(n_blocks):
                block = groups[bi * GROUPS : (bi + 1) * GROUPS]
                nxt = groups[(bi + 1) * GROUPS : (bi + 2) * GROUPS]
                for m, _ in block:
                    transpose_mtile(m)
                if any(qq >= 2 for _, qq in block):
                    ensure_h1_signs()
                psums = {
                    (m, qq): psum_y.tile([P, NF], F32, name=f"ps{qq}_{m}", tag="psy")
                    for m, qq in block
                }
                if bi == n_blocks - 1:
                    # drain block: per-group chains so evictions start early
                    for m, qq in block:
                        for si, st in enumerate(steps):
                            emit_mm(psums[(m, qq)], m, qq, st, si == 0, si == len(steps) - 1)
                        evict(psums[(m, qq)], m, qq)
                    continue
                for si, st in enumerate(steps):
                    for m, qq in block:
                        emit_mm(psums[(m, qq)], m, qq, st, si == 0, si == len(steps) - 1)
                    if si == 2:
                        # early prefetch: next block's transposes (and their
                        # hi8 copies) fill the PE's w-starved stalls
                        for m, _ in nxt:
                            transpose_mtile(m)
                for m, qq in block:
                    evict(psums[(m, qq)], m, qq)

            ensure_h1_signs()
            if __import__("os").environ.get("BASS_V3_DEBUG"):
                xt_dump = nc.dram_tensor("xt_dump", [M_TILES, P, K_TILES, P], BF16, kind="ExternalOutput")
                for m in range(M_TILES):
                    nc.sync.dma_start(xt_dump[m], xts[m][:])
                if kp8:
                    hi_dump = nc.dram_tensor("hi_dump", [M_TILES, P, k8, P], FP8, kind="ExternalOutput")
                    s8_dump = nc.dram_tensor("s8_dump", [2, P, kp8, 2, NH], FP8, kind="ExternalOutput")
                    for m in range(M_TILES):
                        nc.sync.dma_start(hi_dump[m], hi8s[m][:])
                    for h in range(2):
                        nc.sync.dma_start(s8_dump[h], s8s[h][:])

    _split_waits_pass(nc, max_waits=1)
    return nc


def _build_nc():
    import os

    impl = os.environ.get("BASS_DENSE_IMPL", "v3")
    if impl == "f32r":
        return _build_nc_f32r()
    if impl == "fp8":
        return _build_nc_fp8()
    k8 = int(os.environ.get("BASS_K8", "6"))
    return _build_nc_v3(k8=k8)


_NC_CACHE = None


def _get_nc():
    global _NC_CACHE
    if _NC_CACHE is None:
        _NC_CACHE = _build_nc()
    return _NC_CACHE


def _run(inputs, w, b, trace=False):
    nc = _get_nc()
    inputs = np.ascontiguousarray(inputs, dtype=np.float32)
    w = np.ascontiguousarray(w, dtype=np.float32)
    b = np.ascontiguousarray(b, dtype=np.float32)
    in_maps = [
        {"xs": np.ascontiguousarray(inputs[i * ROWS : (i + 1) * ROWS]), "w": w, "b": b}
        for i in range(N_CORES)
    ]
    res = run_bass_kernel_spmd(nc, in_maps, list(range(N_CORES)), trace=trace)
    out = np.concatenate([res.results[i]["y"] for i in range(N_CORES)], axis=0)
    return out, res


def kernel(inputs, w, b):
    out, _ = _run(inputs, w, b, trace=False)
    return out



# revision 17
# speedup vs baseline: 1.0186x; 1.0125x over previous
"""Trainium2 Bass kernel for nn_DenseTf: out = inputs @ sign(clip(w,-1,1)) + b.

Shapes: inputs [8192, 2048] f32, w [2048, 2048] f32, b [2048] f32 -> [8192, 2048] f32.

Sharding: data-parallel over rows. Each of the 8 NeuronCores gets 1024 rows of
`inputs`, plus a full replica of `w` and `b`; no collectives. Outputs are
concatenated on the host.

Default per-core kernel (_build_nc_v3, k8=6; ~146 us on 8 cores, rms rel err
1.64e-2 vs f64 -- gate is 2e-2):
  - x is cast to bf16 (ACT) and transposed on the PE (bf16 identity-matmul
    transposes, 2 batches of 8 per m-tile through 2 PSUM banks, DVE
    eviction) into resident k-major tiles xt[m] [128, 16k, 128m] bf16.
    bf16 quantization of x costs 1.66e-3 rms.
  - w streams as column halves [128, 1024] f32; ACT Sign emits sign(w)
    directly (exact in bf16/fp8). First 10 k-tiles -> bf16 tiles; last
    k8=6 k-tiles -> fp8e4 pair tiles [128, pair, 2, 1024].
  - hi8[m] = fp8e4(xt[m] fp8 k-range) via ACT copy, emitted INSIDE
    transpose_mtile after the xt writers (emitting a read before its
    writer breaks tile dep tracking -> silent garbage).
  - matmuls: 6 PSUM groups (m, quarter) in flight; per group 10 bf16
    matmuls [128k x 512n] at 1 col/cycle plus 3 fp8 DoubleRow matmuls
    [256k x 512n] at the same 213 ns (2x K per instruction = the only
    2x-rate path on TRN2; bf16 hi+lo would cost the same PE time as f32r).
    PE busy ~105 us = 89 us matmul + 17 us transposes.
  - schedule: SP issues the pure input-DMA stream (x m-tiles 1:2 with w
    half-0, then half-1; ~400 GB/s sustained); block 0 chases the streams
    k-major; later blocks prefetch the next block's transposes at step 2;
    the final (full) block drains with per-group chains so evictions
    stagger. y: DVE bias-add then Pool-engine SWDGE writes.

Error dial: k8 in {0,2,4,6} -> rms err {1.7e-3, 9.6e-3, 1.34e-2, 1.64e-2},
PE matmul time {118, 111, 103, 96} us. k8=6 keeps 18% margin to the 2e-2
gate on the fixed harness inputs (verified exactly via test.py).
"""

import numpy as np

import concourse.bass as bass
import concourse.mybir as mybir
import concourse.tile as tile
from concourse.bass_utils import run_bass_kernel_spmd
from concourse.masks import make_identity

N_CORES = 8
N_ROWS, D_IN, D_OUT = 8192, 2048, 2048
ROWS = N_ROWS // N_CORES  # rows per core
P = 128
K_TILES = D_IN // P  # 16
M_TILES = ROWS // P  # 8
NF = 512  # matmul moving free dim / psum bank width (fp32)
N_TILES = D_OUT // NF  # 4
GROUPS_IN_FLIGHT = 6  # concurrent psum accumulation groups (banks)

F32 = mybir.dt.float32
F32R = mybir.dt.float32r


def _split_waits_pass(nc, max_waits=1, zero_types=("InstDmaTransposeAnt",)):
    """Cap semaphore waits per instruction for this container's walrus.

    The pinned walrus errors ("Too many sync wait commands") when an
    instruction carries more than ~2 sync waits, and errors on ANY wait
    attached to the DIRECT2D_XPOSE struct. Move overflow waits onto
    same-engine NoOps inserted immediately before the instruction; the engine
    executes its stream in order, so the gating semantics are identical.
    """
    idx = 0
    for f in nc.m.functions:
        for bb in f.blocks:
            insts = list(bb.instructions)
            changed = False
            out = []
            for inst in insts:
                si = inst.sync_info
                limit = 0 if type(inst).__name__ in zero_types else max_waits
                if si is not None and si.on_wait and len(si.on_wait) > limit:
                    waits = list(si.on_wait)
                    keep, rest = waits[:limit], waits[limit:]
                    for i in range(0, len(rest), max_waits):
                        nop = mybir.InstNoOp(
                            name=f"splitw-{idx}",
                            ins=[],
                            outs=[],
                            engine=inst.engine,
                            sync_info=mybir.SyncInfo(
                                on_wait=rest[i : i + max_waits], on_update=[]
                            ),
                        )
                        idx += 1
                        out.append(nop)
                    inst.sync_info = mybir.SyncInfo(
                        on_wait=keep, on_update=list(si.on_update or [])
                    )
                    changed = True
                out.append(inst)
            if changed:
                bb.instructions.clear()
                bb.instructions.extend(out)


def _build_nc_f32r():
    nc = bass.Bass()
    x_d = nc.dram_tensor("xs", [ROWS, D_IN], F32, kind="ExternalInput")
    w_d = nc.dram_tensor("w", [D_IN, D_OUT], F32, kind="ExternalInput")
    b_d = nc.dram_tensor("b", [D_OUT], F32, kind="ExternalInput")
    y_d = nc.dram_tensor("y", [ROWS, D_OUT], F32, kind="ExternalOutput")

    with tile.TileContext(nc) as tc:
        with (
            tc.tile_pool(name="const", bufs=1) as const,
            tc.tile_pool(name="s", bufs=2 * K_TILES) as s_pool,
            tc.tile_pool(name="wstage", bufs=8) as wstage,
            tc.tile_pool(name="xstage", bufs=2) as xstage,
            tc.tile_pool(name="xt", bufs=3) as xt_pool,
            tc.tile_pool(name="y", bufs=3) as y_pool,
            tc.tile_pool(name="pst", bufs=2, space="PSUM") as psum_t,
            tc.tile_pool(name="psy", bufs=GROUPS_IN_FLIGHT, space="PSUM") as psum_y,
        ):
            ident = const.tile([P, P], F32)
            make_identity(nc, ident)

            # x tiles: DMA natural layout, transpose 128x128 blocks on PE,
            # evict PSUM->SBUF on DVE
            xts = {}

            def ensure_xt(m):
                if m in xts:
                    return
                xa = xstage.tile([P, D_IN], F32, tag="xstage")
                nc.sync.dma_start(xa[:], x_d[m * P : (m + 1) * P, :])
                xt = xt_pool.tile([P, K_TILES, P], F32R, tag="xt")
                for k in range(K_TILES):
                    pt = psum_t.tile([P, P], F32)
                    nc.tensor.transpose(pt[:], xa[:, k * P : (k + 1) * P], ident[:])
                    nc.vector.tensor_copy(xt[:, k, :], pt[:])
                xts[m] = xt

            # front-load the first x tiles so the PE has transpose work (and
            # block0's matmuls can chase the w chunks) during the w load;
            # m2's DMA must queue ahead of the w stream or block1 stalls on it
            for m in range(3):
                ensure_xt(m)

            # sign(w): resident, loaded as half-width chunks so the DMA
            # issue pipeline is not gated on the sign of the chunk two back
            # (wstage bufs=4 at half size = same SBUF, twice the depth)
            HALF = D_OUT // 2
            s_half = {}
            for k in range(K_TILES):
                for h in range(2):
                    wt = wstage.tile([P, HALF], F32, name=f"wt{k}_{h}", tag="wstage")
                    nc.sync.dma_start(
                        wt[:], w_d[k * P : (k + 1) * P, h * HALF : (h + 1) * HALF]
                    )
                    st = s_pool.tile([P, HALF], F32R, name=f"s{k}_{h}", tag="s")
                    nc.scalar.activation(
                        st[:], wt[:], mybir.ActivationFunctionType.Sign
                    )
                    s_half[(k, h)] = st

            # bias: replicate across all 128 partitions via stride-0 DMA
            # (emitted after the x/w loads; first needed ~35us in)
            b_bcast = const.tile([P, D_OUT], F32)
            nc.sync.dma_start(b_bcast[:], b_d[None, :].to_broadcast([P, D_OUT]))

            groups = [(m, n) for m in range(M_TILES) for n in range(N_TILES)]
            for b0 in range(0, len(groups), GROUPS_IN_FLIGHT):
                block = groups[b0 : b0 + GROUPS_IN_FLIGHT]
                for m, _ in block:
                    ensure_xt(m)
                psums = {}
                for m, n in block:
                    psums[(m, n)] = psum_y.tile(
                        [P, NF], F32, name=f"psum_{m}_{n}", tag="psy"
                    )
                for k in range(K_TILES):
                    for m, n in block:
                        nc.tensor.matmul(
                            psums[(m, n)][:],
                            xts[m][:, k, :],
                            s_half[(k, n // 2)][:, (n % 2) * NF : (n % 2 + 1) * NF],
                            start=(k == 0),
                            stop=(k == K_TILES - 1),
                        )
                    if k == 5:
                        # prefetch the next block's x transposes into this
                        # sweep: during the w-load phase the PE is chunk-
                        # starved here, so the transposes fill the stalls
                        # instead of serializing between sweeps
                        for m, _ in groups[b0 + GROUPS_IN_FLIGHT : b0 + 2 * GROUPS_IN_FLIGHT]:
                            ensure_xt(m)
                for m, n in block:
                    yt = y_pool.tile([P, NF], F32)
                    nc.vector.tensor_add(
                        yt[:], psums[(m, n)][:], b_bcast[:, n * NF : (n + 1) * NF]
                    )
                    nc.sync.dma_start(
                        y_d[m * P : (m + 1) * P, n * NF : (n + 1) * NF], yt[:]
                    )

    _split_waits_pass(nc, max_waits=1)
    return nc



FP8 = mybir.dt.float8e4
K_PAIRS = K_TILES // 2  # 8
NHALF = D_OUT // 2  # 1024


def _build_nc_fp8():
    """fp8 DoubleRow kernel: y = x @ sign(w) + b with x = hi + lo (both fp8e4).

    sign(w) is exactly representable in fp8e4, and splitting x into an fp8
    high part plus an fp8 residual keeps the total quantization error ~7e-4
    relative while running the PE at 2x rate (DoubleRow: K=256 per matmul,
    0.5 cyc/row). w is loaded column-half by column-half so the first
    accumulation groups (n in the left half) only gate on 8 MB of w.
    """
    nc = bass.Bass()
    x_d = nc.dram_tensor("xs", [ROWS, D_IN], F32, kind="ExternalInput")
    w_d = nc.dram_tensor("w", [D_IN, D_OUT], F32, kind="ExternalInput")
    b_d = nc.dram_tensor("b", [D_OUT], F32, kind="ExternalInput")
    y_d = nc.dram_tensor("y", [ROWS, D_OUT], F32, kind="ExternalOutput")

    with tile.TileContext(nc) as tc:
        with (
            tc.tile_pool(name="const", bufs=1) as const,
            tc.tile_pool(name="s8", bufs=2 * K_PAIRS) as s_pool,
            tc.tile_pool(name="wstage", bufs=4) as wstage,
            tc.tile_pool(name="xstage", bufs=2) as xstage,
            tc.tile_pool(name="histage", bufs=2) as histage,
            tc.tile_pool(name="lostage", bufs=2) as lostage,
            tc.tile_pool(name="xt8", bufs=2 * M_TILES) as xt_pool,
            tc.tile_pool(name="y", bufs=3) as y_pool,
            tc.tile_pool(name="pst", bufs=2, space="PSUM") as psum_t,
            tc.tile_pool(name="psy", bufs=GROUPS_IN_FLIGHT, space="PSUM") as psum_y,
        ):
            ident8 = const.tile([P, P], FP8)
            make_identity(nc, ident8)
            b_bcast = const.tile([P, D_OUT], F32)
            nc.sync.dma_start(b_bcast[:], b_d[None, :].to_broadcast([P, D_OUT]))

            # x pipeline: load rows, split into fp8 hi + fp8 residual lo,
            # transpose both 128x128-blockwise on the PE (4 blocks per PSUM
            # tile, strided step-2 as the fp8 transpose requires), evict to
            # resident k-major tiles.
            xt_hi = {}
            xt_lo = {}

            def xpipe(m):
                xa = xstage.tile([P, D_IN], F32, name=f"xa{m}", tag="xa")
                nc.sync.dma_start(xa[:], x_d[m * P : (m + 1) * P, :])
                hi8 = histage.tile([P, D_IN], FP8, name=f"hi{m}", tag="hi")
                nc.scalar.copy(hi8[:], xa[:])
                lo8 = lostage.tile([P, D_IN], FP8, name=f"lo{m}", tag="lo")
                nc.vector.tensor_tensor(
                    lo8[:], xa[:], hi8[:], mybir.AluOpType.subtract
                )
                th = xt_pool.tile([P, K_TILES, P], FP8, name=f"xth{m}", tag="xt")
                tl = xt_pool.tile([P, K_TILES, P], FP8, name=f"xtl{m}", tag="xt")
                for src8, dst in ((hi8, th), (lo8, tl)):
                    for q in range(K_TILES // 4):
                        pt = psum_t.tile([P, 4, P, 2], FP8, name=f"pt{m}", tag="pt")
                        for i in range(4):
                            k = 4 * q + i
                            nc.tensor.transpose(
                                pt[:, i, :, 0],
                                src8[:, k * P : (k + 1) * P],
                                ident8[:],
                            )
                        nc.vector.tensor_copy(dst[:, 4 * q : 4 * q + 4, :], pt[:, :, :, 0])
                xt_hi[m] = th
                xt_lo[m] = tl

            # sign(w) per column half: s8[(pair, half)] = [P, 2, NHALF] fp8
            s8 = {}

            def load_w_half(h):
                for k in range(K_TILES):
                    wt = wstage.tile([P, NHALF], F32, name=f"w{h}_{k}", tag="w")
                    nc.sync.dma_start(
                        wt[:], w_d[k * P : (k + 1) * P, h * NHALF : (h + 1) * NHALF]
                    )
                    j = k // 2
                    if (j, h) not in s8:
                        s8[(j, h)] = s_pool.tile(
                            [P, 2, NHALF], FP8, name=f"s{j}_{h}", tag="s"
                        )
                    nc.scalar.activation(
                        s8[(j, h)][:, k % 2, :],
                        wt[:],
                        mybir.ActivationFunctionType.Sign,
                    )

            for m in range(3):
                xpipe(m)
            load_w_half(0)
            for m in range(3, M_TILES):
                xpipe(m)
            load_w_half(1)

            # accumulation groups, left column half first
            groups = [(m, n) for h in range(2) for m in range(M_TILES) for n in (2 * h, 2 * h + 1)]
            for b0 in range(0, len(groups), GROUPS_IN_FLIGHT):
                block = groups[b0 : b0 + GROUPS_IN_FLIGHT]
                psums = {}
                for m, n in block:
                    psums[(m, n)] = psum_y.tile(
                        [P, NF], F32, name=f"psum_{m}_{n}", tag="psy"
                    )
                for pi, xt in enumerate((xt_hi, xt_lo)):
                    for j in range(K_PAIRS):
                        for m, n in block:
                            nc.tensor.matmul(
                                psums[(m, n)][:],
                                xt[m][:, 2 * j : 2 * j + 2, :],
                                s8[(j, n // 2)][:, :, (n % 2) * NF : (n % 2 + 1) * NF],
                                perf_mode=mybir.MatmulPerfMode.DoubleRow,
                                start=(pi == 0 and j == 0),
                                stop=(pi == 1 and j == K_PAIRS - 1),
                            )
                for m, n in block:
                    yt = y_pool.tile([P, NF], F32)
                    nc.vector.tensor_add(
                        yt[:], psums[(m, n)][:], b_bcast[:, n * NF : (n + 1) * NF]
                    )
                    nc.sync.dma_start(
                        y_d[m * P : (m + 1) * P, n * NF : (n + 1) * NF], yt[:]
                    )

    _split_waits_pass(nc, max_waits=1)
    return nc


BF16 = mybir.dt.bfloat16
U16 = mybir.dt.uint16


def _build_nc_v3(k8=0):
    """bf16/fp8 kernel on the proven f32r skeleton (PE transposes, no xbar).

    - x m-tiles: DMA f32 (sync, interleaved with w) -> ACT cast to bf16 ->
      PE transpose in 2 batches of 8 into a PSUM bank -> DVE evict to
      resident k-major bf16 tiles xt[m] [128, 16, 128].
    - w streams as halves [128, 1024] f32; ACT Sign -> resident bf16 tiles
      (and fp8e4 pair tiles for the last k8 k-tiles).
    - matmuls: 6 PSUM groups (m, quarter) in flight (2 banks reserved for
      transposes), k-major in-block; per group kbf bf16 matmuls + kp8 fp8
      DoubleRow matmuls (2x rate).
    - evict: DVE bias-add -> SBUF, y written per (m, quarter) via Pool SWDGE.

    k8 = k-tiles (of 16) in fp8 single-pass; adds ~2.7e-2*sqrt(k8/16) error.
    """
    assert k8 % 2 == 0
    kbf = K_TILES - k8
    kp8 = k8 // 2
    NH = 1024
    GROUPS = 6

    nc = bass.Bass()
    x_d = nc.dram_tensor("xs", [ROWS, D_IN], F32, kind="ExternalInput")
    w_d = nc.dram_tensor("w", [D_IN, D_OUT], F32, kind="ExternalInput")
    b_d = nc.dram_tensor("b", [D_OUT], F32, kind="ExternalInput")
    y_d = nc.dram_tensor("y", [ROWS, D_OUT], F32, kind="ExternalOutput")

    with tile.TileContext(nc) as tc:
        with (
            tc.tile_pool(name="const", bufs=1) as const,
            tc.tile_pool(name="xstage", bufs=3) as xstage,
            tc.tile_pool(name="xbstage", bufs=3) as xbstage,
            tc.tile_pool(name="xt", bufs=M_TILES) as xt_pool,
            tc.tile_pool(name="hi8", bufs=M_TILES) as hi8_pool,
            tc.tile_pool(name="s", bufs=4) as s_pool,
            tc.tile_pool(name="s8", bufs=4) as s8_pool,
            tc.tile_pool(name="wstage", bufs=8) as wstage,
            tc.tile_pool(name="y", bufs=4) as y_pool,
            tc.tile_pool(name="ident", bufs=1) as ident_pool,
            tc.tile_pool(name="pst", bufs=2, space="PSUM") as psum_t,
            tc.tile_pool(name="psy", bufs=GROUPS, space="PSUM") as psum_y,
        ):
            ident = ident_pool.tile([P, P], BF16)
            make_identity(nc, ident)
            b_bcast = const.tile([P, D_OUT], F32)

            # tiles
            xas = [xstage.tile([P, D_IN], F32, name=f"xa{m}", tag="xa") for m in range(M_TILES)]
            xbs = [xbstage.tile([P, D_IN], BF16, name=f"xb{m}", tag="xb") for m in range(M_TILES)]
            xts = [xt_pool.tile([P, K_TILES, P], BF16, name=f"xt{m}", tag="xt") for m in range(M_TILES)]
            s_bfs = [
                s_pool.tile([P, kbf, NF], BF16, name=f"s{q}", tag="s") if kbf else None
                for q in range(4)
            ]
            s8s = [
                s8_pool.tile([P, kp8, 2, NF], FP8, name=f"s8{q}", tag="s8") if kp8 else None
                for q in range(4)
            ]
            hi8s = [
                hi8_pool.tile([P, k8, P], FP8, name=f"hi{m}", tag="hi") if kp8 else None
                for m in range(M_TILES)
            ]
            wts = {}

            # --- SP: pure input DMA stream. x m-tiles split into half DMAs
            # (each transpose batch starts when its half lands), interleaved
            # with w quarter-0 chunks; then quarters 1-3.
            def dma_w(q, k):
                wt = wstage.tile([P, NF], F32, name=f"w{q}_{k}", tag="w")
                nc.sync.dma_start(wt[:], w_d[k * P : (k + 1) * P, q * NF : (q + 1) * NF])
                wts[(q, k)] = wt

            for m in range(M_TILES):
                nc.sync.dma_start(xas[m][:, :NH], x_d[m * P : (m + 1) * P, :NH])
                dma_w(0, 2 * m)
                nc.sync.dma_start(xas[m][:, NH:], x_d[m * P : (m + 1) * P, NH:])
                dma_w(0, 2 * m + 1)
                if m == 2:
                    nc.sync.dma_start(b_bcast[:], b_d[None, :].to_broadcast([P, D_OUT]))
            for q in range(1, 4):
                for k in range(K_TILES):
                    dma_w(q, k)

            # --- ACT: half-casts + signs woven in need-order
            def emit_sign(q, k):
                if k < kbf:
                    nc.scalar.activation(
                        s_bfs[q][:, k, :], wts[(q, k)][:],
                        mybir.ActivationFunctionType.Sign,
                    )
                else:
                    j, par = divmod(k - kbf, 2)
                    nc.scalar.activation(
                        s8s[q][:, j, par, :], wts[(q, k)][:],
                        mybir.ActivationFunctionType.Sign,
                    )

            for m in range(M_TILES):
                nc.scalar.copy(xbs[m][:, :NH], xas[m][:, :NH])
                emit_sign(0, 2 * m)
                nc.scalar.copy(xbs[m][:, NH:], xas[m][:, NH:])
                emit_sign(0, 2 * m + 1)
            # h1 signs are emitted lazily (before the first block that reads
            # them) so the ACT stream serves hi8 copies first; hi8 copies are
            # emitted inside transpose_mtile AFTER the xt writers exist --
            # emitting a read before its writer breaks dep tracking.
            q_signed = [True, False, False, False]

            def ensure_signs(q):
                if q_signed[q]:
                    return
                q_signed[q] = True
                for k in range(K_TILES):
                    emit_sign(q, k)

            # --- PE: transposes (2 batches of 8 per m-tile) woven with
            # matmuls; DVE evictions; Pool y writes.
            transposed = [False] * M_TILES

            def transpose_mtile(m):
                if transposed[m]:
                    return
                transposed[m] = True
                for half in range(2):
                    pt = psum_t.tile([P, 8, P], BF16, name=f"pt{m}_{half}", tag="pt")
                    for i in range(8):
                        k = 8 * half + i
                        nc.tensor.transpose(
                            pt[:, i, :], xbs[m][:, k * P : (k + 1) * P], ident[:]
                        )
                    nc.vector.tensor_copy(xts[m][:, 8 * half : 8 * half + 8, :], pt[:])
                if kp8:
                    nc.scalar.copy(hi8s[m][:], xts[m][:, kbf:, :])

            steps = [("bf", k) for k in range(kbf)] + [("f8", j) for j in range(kp8)]

            def emit_mm(ps, m, qq, step, first, last):
                kind, i = step
                if kind == "bf":
                    nc.tensor.matmul(
                        ps[:], xts[m][:, i, :], s_bfs[qq][:, i, :],
                        start=first, stop=last,
                    )
                else:
                    nc.tensor.matmul(
                        ps[:], hi8s[m][:, 2 * i : 2 * i + 2, :], s8s[qq][:, i, :, :],
                        perf_mode=mybir.MatmulPerfMode.DoubleRow,
                        start=first, stop=last,
                    )

            def evict(ps, m, qq):
                yt = y_pool.tile([P, NF], F32, name=f"yt{qq}_{m}", tag="y")
                nc.vector.tensor_add(yt[:], ps[:], b_bcast[:, qq * NF : (qq + 1) * NF])
                nc.gpsimd.dma_start(
                    y_d[m * P : (m + 1) * P, qq * NF : (qq + 1) * NF], yt[:]
                )

            # front-load first 3 m-tile transposes (they gate block 0)
            for m in range(3):
                transpose_mtile(m)

            groups = [(m, qq) for qq in range(4) for m in range(M_TILES)]
            n_blocks = (len(groups) + GROUPS - 1) // GROUPS
            for bi in range(n_blocks):
                block = groups[bi * GROUPS : (bi + 1) * GROUPS]
                nxt = groups[(bi + 1) * GROUPS : (bi + 2) * GROUPS]
                for m, _ in block:
                    transpose_mtile(m)
                for _, qq in block:
                    ensure_signs(qq)
                for _, qq in nxt:
                    ensure_signs(qq)
                psums = {
                    (m, qq): psum_y.tile([P, NF], F32, name=f"ps{qq}_{m}", tag="psy")
                    for m, qq in block
                }
                if bi == n_blocks - 1:
                    # drain block: per-group chains so evictions start early
                    for m, qq in block:
                        for si, st in enumerate(steps):
                            emit_mm(psums[(m, qq)], m, qq, st, si == 0, si == len(steps) - 1)
                        evict(psums[(m, qq)], m, qq)
                    continue
                for si, st in enumerate(steps):
                    for m, qq in block:
                        emit_mm(psums[(m, qq)], m, qq, st, si == 0, si == len(steps) - 1)
                    if si == 2:
                        # early prefetch: next block's transposes (and their
                        # hi8 copies) fill the PE's w-starved stalls
                        for m, _ in nxt:
                            transpose_mtile(m)
                for m, qq in block:
                    evict(psums[(m, qq)], m, qq)

            for q in range(4):
                ensure_signs(q)
            if __import__("os").environ.get("BASS_V3_DEBUG"):
                xt_dump = nc.dram_tensor("xt_dump", [M_TILES, P, K_TILES, P], BF16, kind="ExternalOutput")
                for m in range(M_TILES):
                    nc.sync.dma_start(xt_dump[m], xts[m][:])
                if kp8:
                    hi_dump = nc.dram_tensor("hi_dump", [M_TILES, P, k8, P], FP8, kind="ExternalOutput")
                    s8_dump = nc.dram_tensor("s8_dump", [4, P, kp8, 2, NF], FP8, kind="ExternalOutput")
                    for m in range(M_TILES):
                        nc.sync.dma_start(hi_dump[m], hi8s[m][:])
                    for q in range(4):
                        nc.sync.dma_start(s8_dump[q], s8s[q][:])

    _split_waits_pass(nc, max_waits=1)
    return nc


def _build_nc():
    import os

    impl = os.environ.get("BASS_DENSE_IMPL", "v3")
    if impl == "f32r":
        return _build_nc_f32r()
    if impl == "fp8":
        return _build_nc_fp8()
    k8 = int(os.environ.get("BASS_K8", "6"))
    return _build_nc_v3(k8=k8)


_NC_CACHE = None


def _get_nc():
    global _NC_CACHE
    if _NC_CACHE is None:
        _NC_CACHE = _build_nc()
    return _NC_CACHE


def _run(inputs, w, b, trace=False):
    nc = _get_nc()
    inputs = np.ascontiguousarray(inputs, dtype=np.float32)
    w = np.ascontiguousarray(w, dtype=np.float32)
    b = np.ascontiguousarray(b, dtype=np.float32)
    in_maps = [
        {"xs": np.ascontiguousarray(inputs[i * ROWS : (i + 1) * ROWS]), "w": w, "b": b}
        for i in range(N_CORES)
    ]
    res = run_bass_kernel_spmd(nc, in_maps, list(range(N_CORES)), trace=trace)
    out = np.concatenate([res.results[i]["y"] for i in range(N_CORES)], axis=0)
    return out, res


def kernel(inputs, w, b):
    out, _ = _run(inputs, w, b, trace=False)
    return out

